# revision 16
# baseline (speedup 1.0000x reference)
"""Single-head cross-attention on 8 NeuronCores, data-parallel over batch.

Math per core (batch element b):
    q = x @ Wq + bq;  k = enc @ Wk + bk;  v = enc @ Wv + bv
    out = softmax(q k^T / sqrt(H)) @ v @ Wp + bp

Weight-fused formulation (exact, host-side folds):
    M   = Wq @ Wk^T / sqrt(H)         [E,E]  host precompute
    Wvp = Wv @ Wp                     [E,E]  host precompute
    scores = x M enc^T (+ row-const from bk: softmax-invariant, dropped;
             + column term from bq: ew[s] = exp(enc[s]@(Wk bq)/sqrt(H)))
    out = (Ex @ [diag(ew)(enc Wvp) | ew]) -> numerator cols 0:768, denom col 768
    bv/bp enter as a host rank-1 add (attn rows sum to 1):  + (bv@Wp + bp)
This drops device work from 4022M MACs to 2816M per core and removes the
separate row-sum/reciprocal-scatter PE phases (denominator rides along as
column 768 of the V operand).

Layout (no on-chip transposes; host pre-tiles everything to the SBUF
partition layout so every DMA moves multi-KB contiguous runs per
partition -- small strided chunks cap HWDGE packet throughput):
    GT[e,t]   = M-tiles as lhsT,   xT as rhs        (= (x@M)^T)
    Vaug[s,:] = encT-tiles as lhsT, Wvp as rhs, scaled by ew[s]; col 768 = ew
    ST[s,t]   = encT-tiles as lhsT, GT as rhs;  Ex = exp(ST)  (no max-sub:
                scores are O(1); softmax shift-invariance keeps it exact)
    O[t,0:769]= Ex-tiles  as lhsT, Vaug as rhs; y = O[:,0:768]/O[:,768]

All matmul operands are bf16 (psum accumulates fp32): same 1 col/cycle PE
rate as fp32r but enables FastWeightLoad so LDWEIGHTS hides under the
streams, and halves DMA bytes. Inputs split across both HWDGE rings (sync
+ scalar); xt ships as two column-halves so the GT h-half-0 matmuls start
as soon as max(mt, xt_h0) lands. The device output is bf16 in partition-
tiled layout, written as 2-tile pairs (3KB contiguous runs) and un-tiled
on host. A short burst of dummy matmuls on a memset tile runs during the
DMA lead-in so the PE HAM clock-gate is already at 2.4 GHz when the real
stream starts. Measured rel err ~5e-3 vs the fp32 reference.
"""

import os

import numpy as np
import ml_dtypes

import concourse.bass as bass
import concourse.bacc as bacc
import concourse.tile as tile
from concourse import mybir
from concourse.bass_utils import run_bass_kernel_spmd

P = 128
B, T, S, E, H = 8, 1024, 1024, 768, 768
NE, NT, NS = E // P, T // P, S // P
VA = E + 1  # Vaug row width: 768 value cols + 1 denominator col
HT = T // 2  # gt column-half (ST stream width)
QT = T // 4  # xt column-quarter (GT pass width)
N_WARMUP = 26  # dummy matmuls to hold the PE busy through the DMA lead-in
# (sized so the warm-up ends ~when xt quarter 0 lands; the idle gap before
#  GT's first real matmul must stay under the ~3.4us HAM re-throttle window)
F32 = mybir.dt.float32
BF16 = mybir.dt.bfloat16
AFT = mybir.ActivationFunctionType
BF16_NP = ml_dtypes.bfloat16

_NC_CACHE = {}
LAST_RESULT = None


def _build_bass():
    nc = bacc.Bacc()
    # xt is stored quarter-major: [P, q*(NE*QT) + j*QT + t'] so each column
    # quarter is one contiguous DMA and GT's first pass waits on only 1/4
    # of the tensor
    xt_d = nc.declare_dram_parameter("xt", [P, NE * T], BF16, isOutput=False)
    encT_d = nc.declare_dram_parameter("encT", [P, NE * S], BF16, isOutput=False)
    mt_d = nc.declare_dram_parameter("mt", [P, NE * E], BF16, isOutput=False)
    wvp_d = nc.declare_dram_parameter("wvp", [P, NE * E], BF16, isOutput=False)
    ew_d = nc.declare_dram_parameter("ew", [P, NS], F32, isOutput=False)
    out_d = nc.declare_dram_parameter("out", [P, NT * E], BF16, isOutput=True)

    def mm(ps, lhsT, rhs, start, stop):
        nc.tensor.matmul(ps, lhsT, rhs, start=start, stop=stop)

    with tile.TileContext(nc) as tc:
        with (
            tc.tile_pool(name="const", bufs=1) as constp,
            tc.tile_pool(name="big", bufs=1) as bigp,
            tc.tile_pool(name="psum", bufs=3, space="PSUM") as psp,
            tc.tile_pool(name="warm", bufs=1, space="PSUM") as warmp,
            tc.tile_pool(name="yout", bufs=2) as youtp,
        ):
            ew_sb = constp.tile([P, NS], F32, tag="ew")
            junk_sb = constp.tile([P, 512], BF16, tag="junk")
            mt_sb = bigp.tile([P, NE * E], BF16, tag="mt")
            xt_sb = bigp.tile([P, NE * T], BF16, tag="xt")
            encT_sb = bigp.tile([P, NE * S], BF16, tag="encT")
            wvp_sb = bigp.tile([P, NE * E], BF16, tag="wvp")
            gt_sb = bigp.tile([P, NE * T], BF16, tag="gt")
            ex_sb = bigp.tile([P, NS * T], BF16, tag="ex")
            vaug_sb = bigp.tile([P, NS * VA], BF16, tag="vaug")

            # inputs ride both HWDGE rings (sync=qSPDynamicHW,
            # scalar=qActDynamicHW) so the two GT operands stream in
            # parallel; one trigger per tensor packet-spreads across all 16
            # SDMA engines of its ring
            # mt rides the sync ring alone (fast first byte); the xt
            # quarters stream down the scalar ring just ahead of GT's
            # consumption. Cross-ring interleaving of the quarters is NOT
            # robust: per-ring throughput varies ~2x run to run and a
            # mis-ordered arrival stalls the PE long enough to re-throttle.
            QX = NE * QT
            nc.sync.dma_start(mt_sb[:], mt_d[:])
            for q in range(4):
                nc.scalar.dma_start(
                    xt_sb[:, q * QX:(q + 1) * QX], xt_d[:, q * QX:(q + 1) * QX])
            nc.sync.dma_start(encT_sb[:], encT_d[:])
            nc.scalar.dma_start(wvp_sb[:], wvp_d[:])
            nc.sync.dma_start(ew_sb[:], ew_d[:])

            # PE warm-up: the HAM clock-gate needs ~3.4us of sustained
            # activity to lift the PE from 1.2 to 2.4 GHz, and re-throttles
            # after ~3.4us idle. Burn the DMA lead-in on junk matmuls so the
            # real stream starts warm.
            nc.vector.memset(junk_sb[:], 0.0)
            wps = warmp.tile([P, 512], F32, tag="warm")
            for _ in range(N_WARMUP):
                mm(wps[:], junk_sb[:, 0:P], junk_sb[:], start=True, stop=True)

            # GT[e-tile i, quarter q] = sum_j M[e_j, e_i]^T @ xT[e_j, tq]
            # (q outer so the first matmul group only needs xt quarter 0;
            #  gt_sb stays half-major for ST's 512-wide streams)
            HX = NE * HT
            for q in range(4):
                for i in range(NE):
                    ps = psp.tile([P, QT], F32, tag="mm")
                    for j in range(NE):
                        mm(ps[:],
                           mt_sb[:, j * E + i * P: j * E + (i + 1) * P],
                           xt_sb[:, q * QX + j * QT: q * QX + (j + 1) * QT],
                           start=(j == 0), stop=(j == NE - 1))
                    dst = (q // 2) * HX + i * HT + (q % 2) * QT
                    if i % 2 == 0:
                        nc.scalar.copy(gt_sb[:, dst:dst + QT], ps[:])
                    else:
                        nc.vector.tensor_copy(gt_sb[:, dst:dst + QT], ps[:])

            # Vaug[s-tile si] = (sum_j encT[e_j, s_si]^T @ Wvp[e_j, :]) * ew
            # (independent of GT/ST -- placed here so its matmuls cover the
            # GT->SBUF copy latency before ST needs gt_sb)
            for si in range(NS):
                ps = psp.tile([P, E], F32, tag="mm")
                for n0, n1 in ((0, 512), (512, E)):
                    for j in range(NE):
                        mm(ps[:, n0:n1],
                           encT_sb[:, j * S + si * P: j * S + (si + 1) * P],
                           wvp_sb[:, j * E + n0: j * E + n1],
                           start=(j == 0), stop=(j == NE - 1))
                nc.vector.tensor_scalar_mul(
                    vaug_sb[:, si * VA: si * VA + E], ps[:], ew_sb[:, si:si + 1])
                nc.vector.tensor_copy(
                    vaug_sb[:, si * VA + E: (si + 1) * VA], ew_sb[:, si:si + 1])

            # ST[s-tile si] = sum_i encT[e_i, s_si]^T @ GT[e_i, :]; Ex = exp
            for si in range(NS):
                ps = psp.tile([P, T], F32, tag="mm")
                for h in (0, 1):
                    for i in range(NE):
                        mm(ps[:, h * HT:(h + 1) * HT],
                           encT_sb[:, i * S + si * P: i * S + (si + 1) * P],
                           gt_sb[:, h * HX + i * HT: h * HX + (i + 1) * HT],
                           start=(i == 0), stop=(i == NE - 1))
                nc.scalar.activation(
                    ex_sb[:, si * T:(si + 1) * T], ps[:], AFT.Exp)

            # O[t-tile ti, 0:769] = sum_si Ex[s_si, t_ti]^T @ Vaug[s_si, :];
            # col 768 is the softmax denominator -- divide, pair two t-tiles
            # per SBUF buffer, DMA pairs (3KB contiguous runs per partition).
            # The last two tiles go out as singles on both rings in parallel
            # so the post-stream drain is one half-pair transfer, not a pair.
            y = None
            for ti in range(NT):
                if ti % 2 == 0 and ti < NT - 2:
                    y = youtp.tile([P, 2 * E], BF16, tag="y")
                elif ti >= NT - 2:
                    y = youtp.tile([P, 2 * E], BF16, tag="y")
                ps = psp.tile([P, T], F32, tag="mm")
                # denominator chunk (cols 512:769, psum bank 1) first: the
                # reciprocal + high-half scaling then overlap the low-half
                # matmuls streaming into bank 0
                for si in range(NS):
                    mm(ps[:, 512:VA],
                       ex_sb[:, si * T + ti * P: si * T + (ti + 1) * P],
                       vaug_sb[:, si * VA + 512: si * VA + VA],
                       start=(si == 0), stop=(si == NS - 1))
                rc = youtp.tile([P, 1], F32, tag="rcp")
                nc.vector.reciprocal(rc[:], ps[:, E:E + 1])
                c0 = (ti % 2) * E if ti < NT - 2 else 0
                nc.scalar.activation(y[:, c0 + 512:c0 + E], ps[:, 512:E],
                                     AFT.Copy, scale=rc[:])
                if ti == NT - 1:
                    # last tile: its high half can leave while the low half
                    # is still streaming -- the post-stream drain is only
                    # the low half's scale + DMA
                    nc.scalar.dma_start(
                        out_d[:, ti * E + 512:(ti + 1) * E], y[:, 512:E])
                for si in range(NS):
                    mm(ps[:, 0:512],
                       ex_sb[:, si * T + ti * P: si * T + (ti + 1) * P],
                       vaug_sb[:, si * VA: si * VA + 512],
                       start=(si == 0), stop=(si == NS - 1))
                nc.scalar.activation(y[:, c0:c0 + 512], ps[:, 0:512],
                                     AFT.Copy, scale=rc[:])
                if ti == NT - 2:
                    nc.sync.dma_start(out_d[:, ti * E:(ti + 1) * E], y[:, 0:E])
                elif ti == NT - 1:
                    nc.sync.dma_start(
                        out_d[:, ti * E: ti * E + 512], y[:, 0:512])
                elif ti % 2 == 1:
                    eng = nc.sync if ti % 4 == 1 else nc.scalar
                    eng.dma_start(out_d[:, (ti - 1) * E:(ti + 1) * E], y[:])
    nc.finalize()
    return nc


def get_nc():
    if "nc" not in _NC_CACHE:
        _NC_CACHE["nc"] = _build_bass()
    return _NC_CACHE["nc"]


def _tile_rows(a, inner):
    """[B, ntiles*P, inner] -> [B, P, ntiles*inner] (SBUF partition layout)."""
    b = a.shape[0]
    nt = a.shape[1] // P
    return np.ascontiguousarray(
        a.reshape(b, nt, P, inner).transpose(0, 2, 1, 3).reshape(b, P, nt * inner))


def kernel(**inputs):
    global LAST_RESULT
    x = np.asarray(inputs["x"], dtype=np.float32)
    enc = np.asarray(inputs["encoder_out"], dtype=np.float32)
    Wq = np.asarray(inputs["Wq"], dtype=np.float32)
    bq = np.asarray(inputs["bq"], dtype=np.float32)
    Wk = np.asarray(inputs["Wk"], dtype=np.float32)
    bk = np.asarray(inputs["bk"], dtype=np.float32)  # noqa: F841  (softmax-invariant)
    Wv = np.asarray(inputs["Wv"], dtype=np.float32)
    bv = np.asarray(inputs["bv"], dtype=np.float32)
    Wp = np.asarray(inputs["Wp"], dtype=np.float32)
    bp = np.asarray(inputs["bp"], dtype=np.float32)

    scale = np.float32(1.0 / np.sqrt(H))
    mt = (Wq @ Wk.T * scale)[None]                     # [1,E,E]
    wvp = (Wv @ Wp)[None]                              # [1,E,E]
    cvec = (bv @ Wp + bp).astype(np.float32)           # exact rank-1 fold
    if bq.any():
        w = (enc @ (Wk @ bq)) * scale                  # [B,S] column term
        ew = np.exp(w, dtype=np.float32)
    else:
        ew = np.ones((B, S), dtype=np.float32)
    ew_in = np.ascontiguousarray(
        ew.reshape(B, NS, P).transpose(0, 2, 1))       # [B,P,NS]
    # xt quarter-major: [B, P, q, j, t'] flattened -- GT's first pass
    # depends only on the first quarter's DMA
    xt = _tile_rows(x.transpose(0, 2, 1), T)           # [B,P,NE*T] f32
    xt = np.ascontiguousarray(
        xt.reshape(B, P, NE, 4, QT).transpose(0, 1, 3, 2, 4)
        .reshape(B, P, NE * T)).astype(BF16_NP)
    encT = _tile_rows(enc.transpose(0, 2, 1), S).astype(BF16_NP)
    mt_t = _tile_rows(mt, E).astype(BF16_NP)[0]
    wvp_t = _tile_rows(wvp, E).astype(BF16_NP)[0]

    nc = get_nc()
    in_maps = [
        {"xt": xt[i], "encT": encT[i], "mt": mt_t, "wvp": wvp_t, "ew": ew_in[i]}
        for i in range(B)
    ]
    res = run_bass_kernel_spmd(
        nc, in_maps, list(range(B)),
        trace=bool(os.environ.get("KERNEL_TRACE")),
    )
    LAST_RESULT = res
    # un-tile the output: [P, ti*E+e] -> [ti*P+p, e]
    dev = np.stack([res.results[i]["out"] for i in range(B)])
    out = dev.reshape(B, P, NT, E).transpose(0, 2, 1, 3).reshape(B, T, E)
    out = out.astype(np.float32)
    if cvec.any():
        out = out + cvec
    return out
